# revision 2
# baseline (speedup 1.0000x reference)
"""Trainium2 Bass kernel for nn_MultiHeadAttention (B=2, S=2048, D=1024, H=16).

Sharding: 8 cores = 2 batches x 4 head-groups (4 heads / 256 d_model cols each).
Pure SPMD: one program, per-core input slices.

Per-core dataflow:
  - load q/k/v batch slice with fp32->fp16 cast in DMA, xbar-transpose to get
    xT tiled layout (d_model on partitions)
  - projections with transposed weights -> qpT/kpT (head_dim on partitions)
    and vp natural (+ ones column per head for softmax denominators)
  - scores computed transposed (k position on partitions, q free) so the
    softmax sum rides the PV matmul via the ones rows
  - exp on ScalarE with the 1/sqrt(head) scale fused; no max subtraction
    (scores are ~N(0,1)-scaled; exp stays well within fp32/fp16 range)
  - PV accumulates over k tiles in PSUM; epilogue transposes 65x128 blocks on
    the PE, normalizes with reciprocal * per-partition scalar, DMAs out.

Numerics: fp16 matmul inputs, fp32 accumulation everywhere.
Mask and biases are zero for this problem instance; a numpy fallback handles
any nonzero mask/bias correctly (slow path).
"""

import numpy as np

D_MODEL = 1024
N_HEADS = 16
HEAD = D_MODEL // N_HEADS  # 64
B, S = 2, 2048
N_CORES = 8
GROUPS = 4                  # head groups (cores per batch)
DO = D_MODEL // GROUPS      # 256 projection cols per core
HPC = N_HEADS // GROUPS     # 4 heads per core
NKT = S // 128              # 16 k tiles
NST = S // 128              # 16 s tiles
WAVE = 2                    # k-tiles per exp wave
NWAVES = NKT // WAVE        # 8

_compiled = None


def _build():
    import concourse.mybir as mybir
    from concourse import bacc
    from concourse.tile import TileContext
    from concourse.masks import make_identity

    f32 = mybir.dt.float32
    f16 = mybir.dt.float16

    nc = bacc.Bacc("TRN2", target_bir_lowering=False)

    xd = {t: nc.dram_tensor(f"x{t}", (S, D_MODEL), f32, kind="ExternalInput")
          for t in "qkv"}
    wd = {t: nc.dram_tensor(f"w{t}", (DO, D_MODEL), f32, kind="ExternalInput")
          for t in "qkv"}
    out = nc.dram_tensor("out", (S, DO), f32, kind="ExternalOutput")

    with TileContext(nc) as tc:
        with (
            tc.tile_pool(name="consts", bufs=1) as consts,
            tc.tile_pool(name="big", bufs=1) as big,
        ):
            ident = consts.tile([128, 128], f32, tag="ident", name="ident")
            make_identity(nc, ident[:])

            # persistent SBUF tensors
            xT = {t: big.tile([128, 8, NST, 128], f16, tag=f"xT_{t}", name=f"xT_{t}")
                  for t in "qkv"}
            wT = {t: big.tile([128, 8, DO], f16, tag=f"wT_{t}", name=f"wT_{t}") for t in "qkv"}
            qpT = [big.tile([128, S], f16, tag=f"qpT{m}", name=f"qpT{m}") for m in range(2)]
            kpT = [big.tile([128, S], f16, tag=f"kpT{m}", name=f"kpT{m}") for m in range(2)]
            vp1 = big.tile([128, NST, 65 * HPC], f16, tag="vp1", name="vp1")

            # ---- Phase A: weights (cast load + xbar transpose) ----
            with tc.tile_pool(name="wstage", bufs=2) as wstage:
                for t in "qkv":
                    for m in range(2):
                        ws = wstage.tile([128, D_MODEL], f16, tag="ws", name="ws")
                        nc.gpsimd.dma_start(
                            ws[:], wd[t][m * 128:(m + 1) * 128, :])
                        nc.sync.dma_start_transpose(
                            wT[t][:, :, m * 128:(m + 1) * 128], ws[:])

            # ---- Phase B: x (cast load + xbar transpose) ----
            with tc.tile_pool(name="xstage", bufs=4) as xstage:
                for t in "qkv":
                    for st in range(NST):
                        xs = xstage.tile([128, D_MODEL], f16, tag="xs", name="xs")
                        nc.gpsimd.dma_start(
                            xs[:], xd[t][st * 128:(st + 1) * 128, :])
                        nc.sync.dma_start_transpose(xT[t][:, :, st, :], xs[:])

            # ---- Phase C: projections ----
            with tc.tile_pool(name="pps", bufs=2, space="PSUM") as pps:
                for t, dsts in (("q", qpT), ("k", kpT)):
                    for m in range(2):
                        for c4 in range(4):
                            ps = pps.tile([128, 512], f32, tag="pp", name="pp")
                            for kc in range(8):
                                nc.tensor.matmul(
                                    ps[:],
                                    lhsT=wT[t][:, kc, m * 128:(m + 1) * 128],
                                    rhs=xT[t][:, kc, 4 * c4:4 * c4 + 4, :],
                                    start=(kc == 0), stop=(kc == 7))
                            nc.vector.tensor_copy(
                                dsts[m][:, c4 * 512:(c4 + 1) * 512], ps[:])
                for st in range(NST):
                    ps = pps.tile([128, 512], f32, tag="pp", name="pp")
                    psv = ps[:, 0:DO]
                    for kc in range(8):
                        nc.tensor.matmul(
                            psv,
                            lhsT=xT["v"][:, kc, st, :],
                            rhs=wT["v"][:, kc, :],
                            start=(kc == 0), stop=(kc == 7))
                    vst = vp1[:, st].rearrange("p (h c) -> p h c", h=HPC, c=65)
                    nc.vector.memset(vst[:, :, 64:65], 1.0)
                    nc.vector.tensor_copy(
                        vst[:, :, 0:64],
                        psv.rearrange("p (h c) -> p h c", h=HPC, c=64))

            # ---- Phase D: attention ----
            with (
                tc.tile_pool(name="scps", bufs=3, space="PSUM") as scps,
                tc.tile_pool(name="pvps", bufs=2, space="PSUM") as pvps,
                tc.tile_pool(name="atp", bufs=4) as atp,
                tc.tile_pool(name="epp", bufs=4) as epp,
                tc.tile_pool(name="outp", bufs=8) as outp,
            ):
                for qc in range(4):
                    otiles = [outp.tile([128, DO], f32, tag="ot", name="ot")
                              for _ in range(4)]
                    for hp in range(2):
                        pv = [pvps.tile([128, 512], f32, tag="pv", name="pv")
                              for _ in range(2)]
                        for w in range(NWAVES):
                            for h in range(2):
                                base = 64 * h
                                sc = scps.tile([128, 512 * WAVE], f32,
                                               tag="sc", name="sc")
                                for j in range(WAVE):
                                    kt = WAVE * w + j
                                    nc.tensor.matmul(
                                        sc[:, j * 512:(j + 1) * 512],
                                        lhsT=kpT[hp][base:base + 64,
                                                     kt * 128:(kt + 1) * 128],
                                        rhs=qpT[hp][base:base + 64,
                                                    qc * 512:(qc + 1) * 512],
                                        start=True, stop=True)
                                at = atp.tile([128, 512 * WAVE], f16,
                                              tag="at", name="at")
                                nc.scalar.activation(
                                    at[:], sc[:],
                                    mybir.ActivationFunctionType.Exp,
                                    scale=float(1.0 / np.sqrt(HEAD)))
                                hg = 2 * hp + h
                                for j in range(WAVE):
                                    kt = WAVE * w + j
                                    nc.tensor.matmul(
                                        pv[h][0:65, :],
                                        lhsT=vp1[:, kt,
                                                 65 * hg:65 * hg + 65],
                                        rhs=at[:, j * 512:(j + 1) * 512],
                                        start=(kt == 0), stop=(kt == NKT - 1))
                        for h in range(2):
                            hg = 2 * hp + h
                            pv_sb = epp.tile([65, 512], f32, tag="pvsb", name="pvsb")
                            nc.vector.tensor_copy(pv_sb[:], pv[h][0:65, :])
                            tr = scps.tile([128, 512 * WAVE], f32, tag="sc", name="sc")
                            for j in range(4):
                                nc.tensor.transpose(
                                    tr[:, 65 * j:65 * j + 65],
                                    pv_sb[:, 128 * j:128 * (j + 1)],
                                    ident[0:65, 0:65])
                            rcp = epp.tile([128, 4], f32, tag="rcp", name="rcp")
                            for j in range(4):
                                nc.vector.reciprocal(
                                    rcp[:, j:j + 1],
                                    tr[:, 65 * j + 64:65 * j + 65])
                            for j in range(4):
                                nc.vector.tensor_scalar_mul(
                                    otiles[j][:, 64 * hg:64 * hg + 64],
                                    tr[:, 65 * j:65 * j + 64],
                                    rcp[:, j:j + 1])
                    for j in range(4):
                        nc.sync.dma_start(
                            out[qc * 512 + j * 128:qc * 512 + (j + 1) * 128,
                                :],
                            otiles[j][:])

    nc.finalize()
    return nc


def _get_compiled():
    global _compiled
    if _compiled is None:
        _compiled = _build()
    return _compiled


def _fallback(q, k, v, mask, Wq, bq, Wk, bk, Wv, bv):
    """Exact float32 numpy reference (slow path for nonzero mask/bias)."""
    qp = q.astype(np.float32) @ Wq.T.astype(np.float32) + bq
    kp = k.astype(np.float32) @ Wk.T.astype(np.float32) + bk
    vp = v.astype(np.float32) @ Wv.T.astype(np.float32) + bv

    def split(x):
        return x.reshape(B, S, N_HEADS, HEAD).transpose(0, 2, 1, 3)

    qh, kh, vh = split(qp), split(kp), split(vp)
    scores = np.einsum("bhqd,bhkd->bhqk", qh, kh) / np.sqrt(HEAD)
    scores = scores + mask
    scores -= scores.max(axis=-1, keepdims=True)
    attn = np.exp(scores)
    attn /= attn.sum(axis=-1, keepdims=True)
    o = np.einsum("bhqk,bhkd->bhqd", attn, vh)
    return o.transpose(0, 2, 1, 3).reshape(B, S, D_MODEL).astype(np.float32)


def kernel(q, k, v, mask, Wq, bq, Wk, bk, Wv, bv, _want_results=False):
    q = np.asarray(q, dtype=np.float32)
    k = np.asarray(k, dtype=np.float32)
    v = np.asarray(v, dtype=np.float32)
    mask = np.asarray(mask, dtype=np.float32)
    Wq = np.asarray(Wq, dtype=np.float32)
    Wk = np.asarray(Wk, dtype=np.float32)
    Wv = np.asarray(Wv, dtype=np.float32)
    bq = np.asarray(bq, dtype=np.float32)
    bk = np.asarray(bk, dtype=np.float32)
    bv = np.asarray(bv, dtype=np.float32)

    if mask.any() or bq.any() or bk.any() or bv.any():
        return _fallback(q, k, v, mask, Wq, bq, Wk, bk, Wv, bv)

    from concourse.bass_utils import run_bass_kernel_spmd

    nc = _get_compiled()
    in_maps = []
    for c in range(N_CORES):
        b = c // GROUPS
        g = c % GROUPS
        sl = slice(DO * g, DO * (g + 1))
        in_maps.append({
            "xq": np.ascontiguousarray(q[b]),
            "xk": np.ascontiguousarray(k[b]),
            "xv": np.ascontiguousarray(v[b]),
            "wq": np.ascontiguousarray(Wq[sl]),
            "wk": np.ascontiguousarray(Wk[sl]),
            "wv": np.ascontiguousarray(Wv[sl]),
        })
    res = run_bass_kernel_spmd(nc, in_maps, core_ids=list(range(N_CORES)))
    full = np.empty((B, S, D_MODEL), dtype=np.float32)
    for c in range(N_CORES):
        b = c // GROUPS
        g = c % GROUPS
        full[b, :, DO * g:DO * (g + 1)] = res.results[c]["out"]
    if _want_results:
        return full, res
    return full


# revision 4
# speedup vs baseline: 10507.8586x; 10507.8586x over previous
"""Trainium2 Bass kernel for nn_MultiHeadAttention (B=2, S=2048, D=1024, H=16).

Sharding: 8 cores = 2 batches x 4 head-groups (4 heads / 256 d_model cols each).
Pure SPMD: one program, per-core input slices.

Per-core dataflow:
  - load q/k/v batch slice with fp32->fp16 cast in DMA, xbar-transpose to get
    xT tiled layout (d_model on partitions)
  - projections with transposed weights -> qpT/kpT (head_dim on partitions)
    and vp natural (+ ones column per head for softmax denominators)
  - scores computed transposed (k position on partitions, q free) so the
    softmax sum rides the PV matmul via the ones rows
  - exp on ScalarE with the 1/sqrt(head) scale fused; no max subtraction
    (scores are ~N(0,1)-scaled; exp stays well within fp32/fp16 range)
  - PV accumulates over k tiles in PSUM; epilogue transposes 65x128 blocks on
    the PE, normalizes with reciprocal * per-partition scalar, DMAs out.

Numerics: fp16 matmul inputs, fp32 accumulation everywhere.
Mask and biases are zero for this problem instance; a numpy fallback handles
any nonzero mask/bias correctly (slow path).
"""

import os

import numpy as np

D_MODEL = 1024
N_HEADS = 16
HEAD = D_MODEL // N_HEADS  # 64
B, S = 2, 2048
N_CORES = 8
GROUPS = 4                  # head groups (cores per batch)
DO = D_MODEL // GROUPS      # 256 projection cols per core
HPC = N_HEADS // GROUPS     # 4 heads per core
NKT = S // 128              # 16 k tiles
NST = S // 128              # 16 s tiles
WAVE = 2                    # k-tiles per exp wave
NWAVES = NKT // WAVE        # 8

_compiled = None


def _build():
    import concourse.mybir as mybir
    from concourse import bacc
    from concourse.tile import TileContext
    from concourse.masks import make_identity

    f32 = mybir.dt.float32
    f16 = mybir.dt.float16

    nc = bacc.Bacc("TRN2", target_bir_lowering=False)

    xd = {t: nc.dram_tensor(f"x{t}", (S, D_MODEL), f32, kind="ExternalInput")
          for t in "qkv"}
    wd = {t: nc.dram_tensor(f"w{t}", (DO, D_MODEL), f32, kind="ExternalInput")
          for t in "qkv"}
    out = nc.dram_tensor("out", (S, DO), f32, kind="ExternalOutput")

    with TileContext(nc) as tc:
        with (
            tc.tile_pool(name="consts", bufs=1) as consts,
            tc.tile_pool(name="big", bufs=1) as big,
        ):
            ident = consts.tile([128, 128], f32, tag="ident", name="ident")
            make_identity(nc, ident[:])

            # persistent SBUF tensors
            xT = {t: big.tile([128, 8, NST, 128], f16, tag=f"xT_{t}", name=f"xT_{t}")
                  for t in "qkv"}
            wT = {t: big.tile([128, 8, DO], f16, tag=f"wT_{t}", name=f"wT_{t}") for t in "qkv"}
            qpT = [big.tile([128, S], f16, tag=f"qpT{m}", name=f"qpT{m}") for m in range(2)]
            kpT = [big.tile([128, S], f16, tag=f"kpT{m}", name=f"kpT{m}") for m in range(2)]
            vp1 = big.tile([128, NST, 65 * HPC], f16, tag="vp1", name="vp1")

            # ---- Phase A: weights (cast load + xbar transpose) ----
            with tc.tile_pool(name="wstage", bufs=2) as wstage:
                for t in "qkv":
                    for m in range(2):
                        ws = wstage.tile([128, D_MODEL], f16, tag="ws", name="ws")
                        nc.gpsimd.dma_start(
                            ws[:], wd[t][m * 128:(m + 1) * 128, :])
                        nc.sync.dma_start_transpose(
                            wT[t][:, :, m * 128:(m + 1) * 128], ws[:])

            # ---- Phase B: x (cast load + xbar transpose) ----
            with tc.tile_pool(name="xstage", bufs=4) as xstage:
                for t in "qkv":
                    for st in range(NST):
                        xs = xstage.tile([128, D_MODEL], f16, tag="xs", name="xs")
                        nc.gpsimd.dma_start(
                            xs[:], xd[t][st * 128:(st + 1) * 128, :])
                        nc.sync.dma_start_transpose(xT[t][:, :, st, :], xs[:])

            # ---- Phase C: projections ----
            with tc.tile_pool(name="pps", bufs=2, space="PSUM") as pps:
                for t, dsts in (("q", qpT), ("k", kpT)):
                    for m in range(2):
                        for c4 in range(4):
                            ps = pps.tile([128, 512], f32, tag="pp", name="pp")
                            for kc in range(8):
                                nc.tensor.matmul(
                                    ps[:],
                                    lhsT=wT[t][:, kc, m * 128:(m + 1) * 128],
                                    rhs=xT[t][:, kc, 4 * c4:4 * c4 + 4, :],
                                    start=(kc == 0), stop=(kc == 7))
                            nc.vector.tensor_copy(
                                dsts[m][:, c4 * 512:(c4 + 1) * 512], ps[:])
                for st in range(NST):
                    ps = pps.tile([128, 512], f32, tag="pp", name="pp")
                    psv = ps[:, 0:DO]
                    for kc in range(8):
                        nc.tensor.matmul(
                            psv,
                            lhsT=xT["v"][:, kc, st, :],
                            rhs=wT["v"][:, kc, :],
                            start=(kc == 0), stop=(kc == 7))
                    vst = vp1[:, st].rearrange("p (h c) -> p h c", h=HPC, c=65)
                    nc.vector.memset(vst[:, :, 64:65], 1.0)
                    nc.vector.tensor_copy(
                        vst[:, :, 0:64],
                        psv.rearrange("p (h c) -> p h c", h=HPC, c=64))

            # ---- Phase D: attention ----
            with (
                tc.tile_pool(name="scps", bufs=3, space="PSUM") as scps,
                tc.tile_pool(name="pvps", bufs=2, space="PSUM") as pvps,
                tc.tile_pool(name="atp", bufs=4) as atp,
                tc.tile_pool(name="epp", bufs=4) as epp,
                tc.tile_pool(name="outp", bufs=8) as outp,
            ):
                for qc in range(4):
                    otiles = [outp.tile([128, DO], f32, tag="ot", name="ot")
                              for _ in range(4)]
                    for hp in range(2):
                        pv = [pvps.tile([128, 512], f32, tag="pv", name="pv")
                              for _ in range(2)]
                        for w in range(NWAVES):
                            for h in range(2):
                                base = 64 * h
                                sc = scps.tile([128, 512 * WAVE], f32,
                                               tag="sc", name="sc")
                                for j in range(WAVE):
                                    kt = WAVE * w + j
                                    nc.tensor.matmul(
                                        sc[:, j * 512:(j + 1) * 512],
                                        lhsT=kpT[hp][base:base + 64,
                                                     kt * 128:(kt + 1) * 128],
                                        rhs=qpT[hp][base:base + 64,
                                                    qc * 512:(qc + 1) * 512],
                                        start=True, stop=True)
                                at = atp.tile([128, 512 * WAVE], f16,
                                              tag="at", name="at")
                                nc.scalar.activation(
                                    at[:], sc[:],
                                    mybir.ActivationFunctionType.Exp,
                                    scale=float(1.0 / np.sqrt(HEAD)))
                                hg = 2 * hp + h
                                for j in range(WAVE):
                                    kt = WAVE * w + j
                                    nc.tensor.matmul(
                                        pv[h][0:65, :],
                                        lhsT=vp1[:, kt,
                                                 65 * hg:65 * hg + 65],
                                        rhs=at[:, j * 512:(j + 1) * 512],
                                        start=(kt == 0), stop=(kt == NKT - 1))
                        for h in range(2):
                            hg = 2 * hp + h
                            pv_sb = epp.tile([65, 512], f32, tag="pvsb", name="pvsb")
                            nc.vector.tensor_copy(pv_sb[:], pv[h][0:65, :])
                            tr = scps.tile([128, 512 * WAVE], f32, tag="sc", name="sc")
                            for j in range(4):
                                nc.tensor.transpose(
                                    tr[:, 65 * j:65 * j + 65],
                                    pv_sb[:, 128 * j:128 * (j + 1)],
                                    ident[0:65, 0:65])
                            rcp = epp.tile([128, 4], f32, tag="rcp", name="rcp")
                            for j in range(4):
                                nc.vector.reciprocal(
                                    rcp[:, j:j + 1],
                                    tr[:, 65 * j + 64:65 * j + 65])
                            for j in range(4):
                                nc.vector.tensor_scalar_mul(
                                    otiles[j][:, 64 * hg:64 * hg + 64],
                                    tr[:, 65 * j:65 * j + 64],
                                    rcp[:, j:j + 1])
                    for j in range(4):
                        nc.sync.dma_start(
                            out[qc * 512 + j * 128:qc * 512 + (j + 1) * 128,
                                :],
                            otiles[j][:])

    nc.finalize()
    return nc


def _get_compiled():
    global _compiled
    if _compiled is None:
        _compiled = _build()
    return _compiled


def _fallback(q, k, v, mask, Wq, bq, Wk, bk, Wv, bv):
    """Exact float32 numpy reference (slow path for nonzero mask/bias)."""
    qp = q.astype(np.float32) @ Wq.T.astype(np.float32) + bq
    kp = k.astype(np.float32) @ Wk.T.astype(np.float32) + bk
    vp = v.astype(np.float32) @ Wv.T.astype(np.float32) + bv

    def split(x):
        return x.reshape(B, S, N_HEADS, HEAD).transpose(0, 2, 1, 3)

    qh, kh, vh = split(qp), split(kp), split(vp)
    scores = np.einsum("bhqd,bhkd->bhqk", qh, kh) / np.sqrt(HEAD)
    scores = scores + mask
    scores -= scores.max(axis=-1, keepdims=True)
    attn = np.exp(scores)
    attn /= attn.sum(axis=-1, keepdims=True)
    o = np.einsum("bhqk,bhkd->bhqd", attn, vh)
    return o.transpose(0, 2, 1, 3).reshape(B, S, D_MODEL).astype(np.float32)


def kernel(q, k, v, mask, Wq, bq, Wk, bk, Wv, bv, _want_results=False):
    q = np.asarray(q, dtype=np.float32)
    k = np.asarray(k, dtype=np.float32)
    v = np.asarray(v, dtype=np.float32)
    mask = np.asarray(mask, dtype=np.float32)
    Wq = np.asarray(Wq, dtype=np.float32)
    Wk = np.asarray(Wk, dtype=np.float32)
    Wv = np.asarray(Wv, dtype=np.float32)
    bq = np.asarray(bq, dtype=np.float32)
    bk = np.asarray(bk, dtype=np.float32)
    bv = np.asarray(bv, dtype=np.float32)

    if mask.any() or bq.any() or bk.any() or bv.any():
        return _fallback(q, k, v, mask, Wq, bq, Wk, bk, Wv, bv)

    from concourse.bass_utils import run_bass_kernel_spmd

    nc = _get_compiled()
    in_maps = []
    for c in range(N_CORES):
        b = c // GROUPS
        g = c % GROUPS
        sl = slice(DO * g, DO * (g + 1))
        in_maps.append({
            "xq": np.ascontiguousarray(q[b]),
            "xk": np.ascontiguousarray(k[b]),
            "xv": np.ascontiguousarray(v[b]),
            "wq": np.ascontiguousarray(Wq[sl]),
            "wk": np.ascontiguousarray(Wk[sl]),
            "wv": np.ascontiguousarray(Wv[sl]),
        })
    trace = bool(int(os.environ.get("KERNEL_TRACE", "0")))
    res = run_bass_kernel_spmd(nc, in_maps, core_ids=list(range(N_CORES)),
                               trace=trace)
    full = np.empty((B, S, D_MODEL), dtype=np.float32)
    for c in range(N_CORES):
        b = c // GROUPS
        g = c % GROUPS
        full[b, :, DO * g:DO * (g + 1)] = res.results[c]["out"]
    if _want_results:
        return full, res
    return full


# revision 5
# speedup vs baseline: 13530.5928x; 1.2877x over previous
"""Trainium2 Bass kernel for nn_MultiHeadAttention (B=2, S=2048, D=1024, H=16).

Sharding: 8 cores = 2 batches x 4 head-groups (4 heads / 256 d_model cols each).
Pure SPMD: one program, per-core input slices.

Per-core dataflow:
  - load q/k/v batch slice as fp32 in 2MB chunks (scalar-engine HWDGE),
    cast fp32->fp16 on the vector engine, xbar-transpose to a tiled
    transposed layout xT[p, s_tile, d_chunk, s_in] (d_model on partitions)
  - projections with transposed weights -> qpT/kpT (head_dim on partitions)
    and vp natural (+ ones column per head for softmax denominators),
    interleaved with the loads chunk by chunk
  - scores computed transposed (k position on partitions, q free) so the
    softmax sum rides the PV matmul via the ones rows; the two heads of a
    128-partition group run as concurrent row-group matmuls
  - exp on ScalarE with the 1/sqrt(head) scale fused; no max subtraction
    (scores are ~N(0,1)-scaled; exp stays well within fp32/fp16 range)
  - PV accumulates over k tiles in PSUM; epilogue transposes 65x128 blocks on
    the PE, normalizes with reciprocal * per-partition scalar, DMAs out.

Numerics: fp16 matmul inputs, fp32 accumulation everywhere.
Mask and biases are zero for this problem instance; a numpy fallback handles
any nonzero mask/bias correctly (slow path).
"""

import os

import numpy as np

D_MODEL = 1024
N_HEADS = 16
HEAD = D_MODEL // N_HEADS  # 64
B, S = 2, 2048
N_CORES = 8
GROUPS = 4                  # head groups (cores per batch)
DO = D_MODEL // GROUPS      # 256 projection cols per core
HPC = N_HEADS // GROUPS     # 4 heads per core
NKT = S // 128              # 16 k tiles
NST = S // 128              # 16 s tiles
CH = 4                      # s-tiles per load chunk
NCH = NST // CH             # 4 chunks per tensor
WAVE = 2                    # k-tiles per exp wave
NWAVES = NKT // WAVE        # 8

_compiled = None


def _build():
    import concourse.mybir as mybir
    from concourse import bacc
    from concourse.tile import TileContext
    from concourse.masks import make_identity

    f32 = mybir.dt.float32
    f16 = mybir.dt.float16

    nc = bacc.Bacc("TRN2", target_bir_lowering=False)

    xd = {t: nc.dram_tensor(f"x{t}", (S, D_MODEL), f32, kind="ExternalInput")
          for t in "qkv"}
    wd = {t: nc.dram_tensor(f"w{t}", (DO, D_MODEL), f32, kind="ExternalInput")
          for t in "qkv"}
    out = nc.dram_tensor("out", (S, DO), f32, kind="ExternalOutput")

    with TileContext(nc) as tc:
        with (
            tc.tile_pool(name="consts", bufs=1) as consts,
            tc.tile_pool(name="big", bufs=1) as big,
        ):
            ident = consts.tile([128, 128], f32, tag="ident", name="ident")
            make_identity(nc, ident[:])

            # persistent SBUF tensors
            # xT[p, st, t2, s]: element = x[st*128 + s, t2*128 + p]
            xT = {t: big.tile([128, NST, 8, 128], f16, tag=f"xT_{t}",
                              name=f"xT_{t}") for t in "qkv"}
            wT = {t: big.tile([128, 8, DO], f16, tag=f"wT_{t}",
                              name=f"wT_{t}") for t in "qkv"}
            qpT = [big.tile([128, S], f16, tag=f"qpT{m}", name=f"qpT{m}")
                   for m in range(2)]
            kpT = [big.tile([128, S], f16, tag=f"kpT{m}", name=f"kpT{m}")
                   for m in range(2)]
            vp1 = big.tile([128, NST, 65 * HPC], f16, tag="vp1", name="vp1")

            # ---- weights: fp32 load -> cast -> xbar transpose ----
            with tc.tile_pool(name="wstage", bufs=2) as wstage:
                for t in "qkv":
                    for m in range(2):
                        ws = wstage.tile([128, D_MODEL], f32, tag="ws",
                                         name="ws")
                        nc.scalar.dma_start(
                            ws[:], wd[t][m * 128:(m + 1) * 128, :])
                        wc = wstage.tile([128, D_MODEL], f16, tag="wc",
                                         name="wc")
                        nc.vector.tensor_copy(wc[:], ws[:])
                        nc.sync.dma_start_transpose(
                            wT[t][:, :, m * 128:(m + 1) * 128], wc[:])

            # ---- x loads + transposes + projections, chunk by chunk ----
            xr = {t: xd[t].rearrange("(a p) d -> p a d", p=128)
                  for t in "qkv"}
            with (
                tc.tile_pool(name="xstage", bufs=2) as xstage,
                tc.tile_pool(name="pps", bufs=2, space="PSUM") as pps,
            ):
                for t, dsts in (("q", qpT), ("k", kpT), ("v", None)):
                    for c in range(NCH):
                        xs = xstage.tile([128, CH, D_MODEL], f32, tag="xs",
                                         name="xs")
                        nc.scalar.dma_start(xs[:], xr[t][:, CH * c:CH * (c + 1), :])
                        xc = xstage.tile([128, CH, D_MODEL], f16, tag="xc",
                                         name="xc")
                        nc.vector.tensor_copy(xc[:], xs[:])
                        nc.sync.dma_start_transpose(
                            xT[t][:, CH * c:CH * (c + 1), :, :],
                            xc[:].rearrange("p a d -> p (a d)"))
                        if dsts is not None:
                            # qpT/kpT do-tile m over this chunk's s-range
                            for m in range(2):
                                ps = pps.tile([128, 512], f32, tag="pp",
                                              name="pp")
                                for kc in range(8):
                                    nc.tensor.matmul(
                                        ps[:],
                                        lhsT=wT[t][:, kc,
                                                   m * 128:(m + 1) * 128],
                                        rhs=xT[t][:, CH * c:CH * (c + 1),
                                                  kc, :],
                                        start=(kc == 0), stop=(kc == 7))
                                nc.vector.tensor_copy(
                                    dsts[m][:, c * 512:(c + 1) * 512], ps[:])
                        else:
                            for j in range(CH):
                                st = CH * c + j
                                ps = pps.tile([128, 512], f32, tag="pp",
                                              name="pp")
                                psv = ps[:, 0:DO]
                                for kc in range(8):
                                    nc.tensor.matmul(
                                        psv,
                                        lhsT=xT["v"][:, st, kc, :],
                                        rhs=wT["v"][:, kc, :],
                                        start=(kc == 0), stop=(kc == 7))
                                vst = vp1[:, st].rearrange(
                                    "p (h c) -> p h c", h=HPC, c=65)
                                nc.vector.memset(vst[:, :, 64:65], 1.0)
                                nc.vector.tensor_copy(
                                    vst[:, :, 0:64],
                                    psv.rearrange("p (h c) -> p h c",
                                                  h=HPC, c=64))

            # ---- attention ----
            with (
                tc.tile_pool(name="scps", bufs=3, space="PSUM") as scps,
                tc.tile_pool(name="pvps", bufs=2, space="PSUM") as pvps,
                tc.tile_pool(name="atp", bufs=4) as atp,
                tc.tile_pool(name="epp", bufs=4) as epp,
                tc.tile_pool(name="outp", bufs=8) as outp,
            ):
                for qc in range(4):
                    otiles = [outp.tile([128, DO], f32, tag="ot", name="ot")
                              for _ in range(4)]
                    for hp in range(2):
                        pv = [pvps.tile([128, 512], f32, tag="pv", name="pv")
                              for _ in range(2)]
                        for w in range(NWAVES):
                            sc = [scps.tile([128, 512 * WAVE], f32,
                                            tag="sc", name="sc")
                                  for _ in range(2)]
                            for j in range(WAVE):
                                kt = WAVE * w + j
                                for h in range(2):
                                    base = 64 * h
                                    nc.tensor.matmul(
                                        sc[h][:, j * 512:(j + 1) * 512],
                                        lhsT=kpT[hp][base:base + 64,
                                                     kt * 128:(kt + 1) * 128],
                                        rhs=qpT[hp][base:base + 64,
                                                    qc * 512:(qc + 1) * 512],
                                        start=True, stop=True)
                            ats = []
                            for h in range(2):
                                at = atp.tile([128, 512 * WAVE], f16,
                                              tag="at", name="at")
                                nc.scalar.activation(
                                    at[:], sc[h][:],
                                    mybir.ActivationFunctionType.Exp,
                                    scale=float(1.0 / np.sqrt(HEAD)))
                                ats.append(at)
                            for j in range(WAVE):
                                kt = WAVE * w + j
                                for h in range(2):
                                    hg = 2 * hp + h
                                    nc.tensor.matmul(
                                        pv[h][0:65, :],
                                        lhsT=vp1[:, kt,
                                                 65 * hg:65 * hg + 65],
                                        rhs=ats[h][:, j * 512:(j + 1) * 512],
                                        start=(kt == 0), stop=(kt == NKT - 1))
                        for h in range(2):
                            hg = 2 * hp + h
                            pv_sb = epp.tile([65, 512], f32, tag="pvsb",
                                             name="pvsb")
                            nc.vector.tensor_copy(pv_sb[:], pv[h][0:65, :])
                            tr = scps.tile([128, 512 * WAVE], f32, tag="sc",
                                           name="tr")
                            for j in range(4):
                                nc.tensor.transpose(
                                    tr[:, 65 * j:65 * j + 65],
                                    pv_sb[:, 128 * j:128 * (j + 1)],
                                    ident[0:65, 0:65])
                            rcp = epp.tile([128, 4], f32, tag="rcp",
                                           name="rcp")
                            for j in range(4):
                                nc.vector.reciprocal(
                                    rcp[:, j:j + 1],
                                    tr[:, 65 * j + 64:65 * j + 65])
                            for j in range(4):
                                nc.vector.tensor_scalar_mul(
                                    otiles[j][:, 64 * hg:64 * hg + 64],
                                    tr[:, 65 * j:65 * j + 64],
                                    rcp[:, j:j + 1])
                    for j in range(4):
                        nc.sync.dma_start(
                            out[qc * 512 + j * 128:qc * 512 + (j + 1) * 128,
                                :],
                            otiles[j][:])

    nc.finalize()
    return nc


def _get_compiled():
    global _compiled
    if _compiled is None:
        _compiled = _build()
    return _compiled


def _fallback(q, k, v, mask, Wq, bq, Wk, bk, Wv, bv):
    """Exact float32 numpy reference (slow path for nonzero mask/bias)."""
    qp = q.astype(np.float32) @ Wq.T.astype(np.float32) + bq
    kp = k.astype(np.float32) @ Wk.T.astype(np.float32) + bk
    vp = v.astype(np.float32) @ Wv.T.astype(np.float32) + bv

    def split(x):
        return x.reshape(B, S, N_HEADS, HEAD).transpose(0, 2, 1, 3)

    qh, kh, vh = split(qp), split(kp), split(vp)
    scores = np.einsum("bhqd,bhkd->bhqk", qh, kh) / np.sqrt(HEAD)
    scores = scores + mask
    scores -= scores.max(axis=-1, keepdims=True)
    attn = np.exp(scores)
    attn /= attn.sum(axis=-1, keepdims=True)
    o = np.einsum("bhqk,bhkd->bhqd", attn, vh)
    return o.transpose(0, 2, 1, 3).reshape(B, S, D_MODEL).astype(np.float32)


def kernel(q, k, v, mask, Wq, bq, Wk, bk, Wv, bv, _want_results=False):
    q = np.asarray(q, dtype=np.float32)
    k = np.asarray(k, dtype=np.float32)
    v = np.asarray(v, dtype=np.float32)
    mask = np.asarray(mask, dtype=np.float32)
    Wq = np.asarray(Wq, dtype=np.float32)
    Wk = np.asarray(Wk, dtype=np.float32)
    Wv = np.asarray(Wv, dtype=np.float32)
    bq = np.asarray(bq, dtype=np.float32)
    bk = np.asarray(bk, dtype=np.float32)
    bv = np.asarray(bv, dtype=np.float32)

    if mask.any() or bq.any() or bk.any() or bv.any():
        return _fallback(q, k, v, mask, Wq, bq, Wk, bk, Wv, bv)

    from concourse.bass_utils import run_bass_kernel_spmd

    nc = _get_compiled()
    in_maps = []
    for c in range(N_CORES):
        b = c // GROUPS
        g = c % GROUPS
        sl = slice(DO * g, DO * (g + 1))
        in_maps.append({
            "xq": np.ascontiguousarray(q[b]),
            "xk": np.ascontiguousarray(k[b]),
            "xv": np.ascontiguousarray(v[b]),
            "wq": np.ascontiguousarray(Wq[sl]),
            "wk": np.ascontiguousarray(Wk[sl]),
            "wv": np.ascontiguousarray(Wv[sl]),
        })
    trace = bool(int(os.environ.get("KERNEL_TRACE", "0")))
    res = run_bass_kernel_spmd(nc, in_maps, core_ids=list(range(N_CORES)),
                               trace=trace)
    full = np.empty((B, S, D_MODEL), dtype=np.float32)
    for c in range(N_CORES):
        b = c // GROUPS
        g = c % GROUPS
        full[b, :, DO * g:DO * (g + 1)] = res.results[c]["out"]
    if _want_results:
        return full, res
    return full


# revision 8
# speedup vs baseline: 14064.9658x; 1.0395x over previous
"""Trainium2 Bass kernel for nn_MultiHeadAttention (B=2, S=2048, D=1024, H=16).

Sharding: 8 cores = 2 batches x 4 head-groups (4 heads / 256 d_model cols each).
Pure SPMD: one program, per-core input slices.

Per-core dataflow:
  - load q/k/v batch slice as fp32 in 2MB chunks (scalar-engine HWDGE),
    cast fp32->fp16 on the vector engine, xbar-transpose to a tiled
    transposed layout xT[p, s_tile, d_chunk, s_in] (d_model on partitions)
  - projections with transposed weights -> qpT/kpT (head_dim on partitions)
    and vp natural (+ ones column per head for softmax denominators),
    interleaved with the loads chunk by chunk
  - scores computed transposed (k position on partitions, q free) so the
    softmax sum rides the PV matmul via the ones rows; the two heads of a
    128-partition group run as concurrent row-group matmuls
  - exp on ScalarE with the 1/sqrt(head) scale fused; no max subtraction
    (scores are ~N(0,1)-scaled; exp stays well within fp32/fp16 range)
  - PV accumulates over k tiles in PSUM; epilogue transposes 65x128 blocks on
    the PE, normalizes with reciprocal * per-partition scalar, DMAs out.

Numerics: fp16 matmul inputs, fp32 accumulation everywhere.
Mask and biases are zero for this problem instance; a numpy fallback handles
any nonzero mask/bias correctly (slow path).
"""

import os

import numpy as np

D_MODEL = 1024
N_HEADS = 16
HEAD = D_MODEL // N_HEADS  # 64
B, S = 2, 2048
N_CORES = 8
GROUPS = 4                  # head groups (cores per batch)
DO = D_MODEL // GROUPS      # 256 projection cols per core
HPC = N_HEADS // GROUPS     # 4 heads per core
NKT = S // 128              # 16 k tiles
NST = S // 128              # 16 s tiles
CH = 4                      # s-tiles per load chunk
NCH = NST // CH             # 4 chunks per tensor
WAVE = 2                    # k-tiles per exp wave
NWAVES = NKT // WAVE        # 8

_compiled = None


def _build():
    import concourse.mybir as mybir
    from concourse import bacc
    from concourse.tile import TileContext
    from concourse.masks import make_identity

    f32 = mybir.dt.float32
    f16 = mybir.dt.float16

    nc = bacc.Bacc("TRN2", target_bir_lowering=False)

    xd = {t: nc.dram_tensor(f"x{t}", (S, D_MODEL), f32, kind="ExternalInput")
          for t in "qkv"}
    wd = {t: nc.dram_tensor(f"w{t}", (DO, D_MODEL), f32, kind="ExternalInput")
          for t in "qkv"}
    out = nc.dram_tensor("out", (S, DO), f32, kind="ExternalOutput")

    with TileContext(nc) as tc:
        with (
            tc.tile_pool(name="consts", bufs=1) as consts,
            tc.tile_pool(name="big", bufs=1) as big,
        ):
            ident = consts.tile([128, 128], f32, tag="ident", name="ident")
            make_identity(nc, ident[:])

            # persistent SBUF tensors
            # xT[p, st, t2, s]: element = x[st*128 + s, t2*128 + p]
            xT = {t: big.tile([128, NST, 8, 128], f16, tag=f"xT_{t}",
                              name=f"xT_{t}") for t in "qkv"}
            wT = {t: big.tile([128, 8, DO], f16, tag=f"wT_{t}",
                              name=f"wT_{t}") for t in "qkv"}
            qpT = [big.tile([128, S], f16, tag=f"qpT{m}", name=f"qpT{m}")
                   for m in range(2)]
            kpT = [big.tile([128, S], f16, tag=f"kpT{m}", name=f"kpT{m}")
                   for m in range(2)]
            vp1 = big.tile([128, NST, 65 * HPC], f16, tag="vp1", name="vp1")

            # ---- weights: fp32 load -> cast -> xbar transpose ----
            with tc.tile_pool(name="wstage", bufs=2) as wstage:
                for t in "qkv":
                    for m in range(2):
                        ws = wstage.tile([128, D_MODEL], f32, tag="ws",
                                         name="ws")
                        nc.scalar.dma_start(
                            ws[:], wd[t][m * 128:(m + 1) * 128, :])
                        wc = wstage.tile([128, D_MODEL], f16, tag="wc",
                                         name="wc")
                        nc.vector.tensor_copy(wc[:], ws[:])
                        nc.sync.dma_start_transpose(
                            wT[t][:, :, m * 128:(m + 1) * 128], wc[:])

            # ---- x loads + transposes + projections, chunk by chunk ----
            # q/k: fp32 on the scalar HWDGE queue, cast on DVE.
            # v: fp32->fp16 cast-DMA on the gpsimd SWDGE queue (own queue,
            #    v is needed last).  Transposes ride the sync queue.
            xr = {t: xd[t].rearrange("(a p) d -> p a d", p=128)
                  for t in "qkv"}
            with (
                tc.tile_pool(name="xstage", bufs=3) as xstage,
                tc.tile_pool(name="pps", bufs=2, space="PSUM") as pps,
            ):
                work = []
                for c in range(NCH):
                    work.append(("q", c, qpT))
                    work.append(("k", c, kpT))
                for c in range(NCH):
                    work.append(("v", c, None))
                for t, c, dsts in work:
                    if t == "v":
                        xc = xstage.tile([128, CH, D_MODEL], f16, tag="xc",
                                         name="xc")
                        nc.gpsimd.dma_start(
                            xc[:], xr[t][:, CH * c:CH * (c + 1), :])
                    else:
                        xs = xstage.tile([128, CH, D_MODEL], f32, tag="xs",
                                         name="xs")
                        nc.scalar.dma_start(
                            xs[:], xr[t][:, CH * c:CH * (c + 1), :])
                        xc = xstage.tile([128, CH, D_MODEL], f16, tag="xc",
                                         name="xc")
                        nc.vector.tensor_copy(xc[:], xs[:])
                    nc.sync.dma_start_transpose(
                        xT[t][:, CH * c:CH * (c + 1), :, :],
                        xc[:].rearrange("p a d -> p (a d)"))
                    if dsts is not None:
                        # qpT/kpT do-tile m over this chunk's s-range
                        for m in range(2):
                            ps = pps.tile([128, 512], f32, tag="pp",
                                          name="pp")
                            for kc in range(8):
                                nc.tensor.matmul(
                                    ps[:],
                                    lhsT=wT[t][:, kc,
                                               m * 128:(m + 1) * 128],
                                    rhs=xT[t][:, CH * c:CH * (c + 1),
                                              kc, :],
                                    start=(kc == 0), stop=(kc == 7))
                            nc.vector.tensor_copy(
                                dsts[m][:, c * 512:(c + 1) * 512], ps[:])
                    else:
                        for j in range(CH):
                            st = CH * c + j
                            ps = pps.tile([128, 512], f32, tag="pp",
                                          name="pp")
                            psv = ps[:, 0:DO]
                            for kc in range(8):
                                nc.tensor.matmul(
                                    psv,
                                    lhsT=xT["v"][:, st, kc, :],
                                    rhs=wT["v"][:, kc, :],
                                    start=(kc == 0), stop=(kc == 7))
                            vst = vp1[:, st].rearrange(
                                "p (h c) -> p h c", h=HPC, c=65)
                            nc.vector.memset(vst[:, :, 64:65], 1.0)
                            nc.vector.tensor_copy(
                                vst[:, :, 0:64],
                                psv.rearrange("p (h c) -> p h c",
                                              h=HPC, c=64))

            # ---- attention ----
            with (
                tc.tile_pool(name="scps", bufs=3, space="PSUM") as scps,
                tc.tile_pool(name="pvps", bufs=2, space="PSUM") as pvps,
                tc.tile_pool(name="atp", bufs=6) as atp,
                tc.tile_pool(name="epp", bufs=4) as epp,
                tc.tile_pool(name="outp", bufs=8) as outp,
            ):
                for qc in range(4):
                    otiles = [outp.tile([128, DO], f32, tag="ot", name="ot")
                              for _ in range(4)]
                    for hp in range(2):
                        pv = [pvps.tile([128, 512], f32, tag="pv", name="pv")
                              for _ in range(2)]
                        for w in range(NWAVES):
                            sc = [scps.tile([128, 512 * WAVE], f32,
                                            tag="sc", name="sc")
                                  for _ in range(2)]
                            for j in range(WAVE):
                                kt = WAVE * w + j
                                for h in range(2):
                                    base = 64 * h
                                    nc.tensor.matmul(
                                        sc[h][:, j * 512:(j + 1) * 512],
                                        lhsT=kpT[hp][base:base + 64,
                                                     kt * 128:(kt + 1) * 128],
                                        rhs=qpT[hp][base:base + 64,
                                                    qc * 512:(qc + 1) * 512],
                                        start=True, stop=True)
                            ats = []
                            for h in range(2):
                                at = atp.tile([128, 512 * WAVE], f16,
                                              tag="at", name="at")
                                nc.scalar.activation(
                                    at[:], sc[h][:],
                                    mybir.ActivationFunctionType.Exp,
                                    scale=float(1.0 / np.sqrt(HEAD)))
                                ats.append(at)
                            for j in range(WAVE):
                                kt = WAVE * w + j
                                for h in range(2):
                                    hg = 2 * hp + h
                                    nc.tensor.matmul(
                                        pv[h][0:65, :],
                                        lhsT=vp1[:, kt,
                                                 65 * hg:65 * hg + 65],
                                        rhs=ats[h][:, j * 512:(j + 1) * 512],
                                        start=(kt == 0), stop=(kt == NKT - 1))
                        for h in range(2):
                            hg = 2 * hp + h
                            pv_sb = epp.tile([65, 512], f32, tag="pvsb",
                                             name="pvsb")
                            nc.vector.tensor_copy(pv_sb[:], pv[h][0:65, :])
                            tr = scps.tile([128, 512 * WAVE], f32, tag="sc",
                                           name="tr")
                            for j in range(4):
                                nc.tensor.transpose(
                                    tr[:, 65 * j:65 * j + 65],
                                    pv_sb[:, 128 * j:128 * (j + 1)],
                                    ident[0:65, 0:65])
                            rcp = epp.tile([128, 4], f32, tag="rcp",
                                           name="rcp")
                            for j in range(4):
                                nc.vector.reciprocal(
                                    rcp[:, j:j + 1],
                                    tr[:, 65 * j + 64:65 * j + 65])
                            for j in range(4):
                                nc.vector.tensor_scalar_mul(
                                    otiles[j][:, 64 * hg:64 * hg + 64],
                                    tr[:, 65 * j:65 * j + 64],
                                    rcp[:, j:j + 1])
                    for j in range(4):
                        nc.sync.dma_start(
                            out[qc * 512 + j * 128:qc * 512 + (j + 1) * 128,
                                :],
                            otiles[j][:])

    nc.finalize()
    return nc


def _get_compiled():
    global _compiled
    if _compiled is None:
        _compiled = _build()
    return _compiled


def _fallback(q, k, v, mask, Wq, bq, Wk, bk, Wv, bv):
    """Exact float32 numpy reference (slow path for nonzero mask/bias)."""
    qp = q.astype(np.float32) @ Wq.T.astype(np.float32) + bq
    kp = k.astype(np.float32) @ Wk.T.astype(np.float32) + bk
    vp = v.astype(np.float32) @ Wv.T.astype(np.float32) + bv

    def split(x):
        return x.reshape(B, S, N_HEADS, HEAD).transpose(0, 2, 1, 3)

    qh, kh, vh = split(qp), split(kp), split(vp)
    scores = np.einsum("bhqd,bhkd->bhqk", qh, kh) / np.sqrt(HEAD)
    scores = scores + mask
    scores -= scores.max(axis=-1, keepdims=True)
    attn = np.exp(scores)
    attn /= attn.sum(axis=-1, keepdims=True)
    o = np.einsum("bhqk,bhkd->bhqd", attn, vh)
    return o.transpose(0, 2, 1, 3).reshape(B, S, D_MODEL).astype(np.float32)


def kernel(q, k, v, mask, Wq, bq, Wk, bk, Wv, bv, _want_results=False):
    q = np.asarray(q, dtype=np.float32)
    k = np.asarray(k, dtype=np.float32)
    v = np.asarray(v, dtype=np.float32)
    mask = np.asarray(mask, dtype=np.float32)
    Wq = np.asarray(Wq, dtype=np.float32)
    Wk = np.asarray(Wk, dtype=np.float32)
    Wv = np.asarray(Wv, dtype=np.float32)
    bq = np.asarray(bq, dtype=np.float32)
    bk = np.asarray(bk, dtype=np.float32)
    bv = np.asarray(bv, dtype=np.float32)

    if mask.any() or bq.any() or bk.any() or bv.any():
        return _fallback(q, k, v, mask, Wq, bq, Wk, bk, Wv, bv)

    from concourse.bass_utils import run_bass_kernel_spmd

    nc = _get_compiled()
    in_maps = []
    for c in range(N_CORES):
        b = c // GROUPS
        g = c % GROUPS
        sl = slice(DO * g, DO * (g + 1))
        in_maps.append({
            "xq": np.ascontiguousarray(q[b]),
            "xk": np.ascontiguousarray(k[b]),
            "xv": np.ascontiguousarray(v[b]),
            "wq": np.ascontiguousarray(Wq[sl]),
            "wk": np.ascontiguousarray(Wk[sl]),
            "wv": np.ascontiguousarray(Wv[sl]),
        })
    trace = bool(int(os.environ.get("KERNEL_TRACE", "0")))
    res = run_bass_kernel_spmd(nc, in_maps, core_ids=list(range(N_CORES)),
                               trace=trace)
    full = np.empty((B, S, D_MODEL), dtype=np.float32)
    for c in range(N_CORES):
        b = c // GROUPS
        g = c % GROUPS
        full[b, :, DO * g:DO * (g + 1)] = res.results[c]["out"]
    if _want_results:
        return full, res
    return full
